# revision 25
# baseline (speedup 1.0000x reference)
"""Trainium2 Bass kernel for nn_LocalMixer: grouped 16x16 mixing conv.

out[b, h, t*16+go] = sum_gi W[h, go, gi] * x[b, h, t*16+gi]

Scheme: shard HIDDEN across the 8 cores (64 channels each, all 256
batches). The kernel is HBM-bound, so all layout work is on the host
and all device I/O is fp16:

  - x uploaded PRE-TRANSPOSED fp16: xt[p, h, sh, b] = x[b, h, sh*128+p].
    Partition p = seq pos within a 128-half = (t8, gi), exactly the PE
    contraction layout -> no on-device transposes.
  - W uploaded COMPACT fp16 (wc[gi, h, go], 32 KB) and kron-expanded on
    device into wk[p=(t8,gi), h, (t8',go)] = I8(t8,t8') * W[h,go,gi]
    via memset + 8 strided copies -- keeps 2.1 MB of block-diag zeros
    off HBM.
  - Per h: ONE matmul, lhsT = wk[:, h, :] (stationary 128x128 kron
    block), rhs = xt[:, h, :, :] (all 512 (sh, b) columns stream) ->
    PSUM [(t8, go), (sh, b)] f32.
  - PSUM -> fp16 SBUF copy (vector/scalar alternating), stored to yt in
    the transposed layout yt[p, h, sh, b]; the host un-transposes and
    upcasts to f32 for free.

Device traffic: 8.39 (in) + 8.39 (out) + 0.03 (W) MB per core at fp16,
vs 33.6 MB for the f32 version. Rel err ~3e-4 (fp16 operand rounding;
f32 PSUM accumulate).
"""

import numpy as np

B = 256
HIDDEN = 512
BLOCK = 16
GROUPS = 16
SEQ = BLOCK * GROUPS  # 256
N_CORES = 8
H_CORE = HIDDEN // N_CORES  # 64 hidden channels per core
HSL = 16  # h channels per input/output slice (16 KiB DMA rows)
NSL = H_CORE // HSL  # 4 slices

_cached = None


def _build_bass():
    import concourse.mybir as mybir
    from concourse import bacc
    from concourse.tile import TileContext

    f32 = mybir.dt.float32
    f16 = mybir.dt.float16
    nc = bacc.Bacc()
    # xt layout: [p, (h, sh, b)] with p = seq pos within 128-half
    xt = nc.declare_dram_parameter("xt", [128, H_CORE * 2 * B], f16, isOutput=False)
    # pair-expanded weights: wc[p2, (h, c2)] = kron(I2, W[h].T)
    wc = nc.declare_dram_parameter("wc", [32, H_CORE * 32], f16, isOutput=False)
    # yt layout: [p, (h, sh, b)] like xt; host un-transposes
    yt = nc.declare_dram_parameter("yt", [128, H_CORE * 2 * B], f16, isOutput=True)

    with TileContext(nc) as tc:
        with (
            tc.tile_pool(name="wpool", bufs=1) as wpool,
            tc.tile_pool(name="wcpool", bufs=1) as wcpool,
            tc.tile_pool(name="natpool", bufs=NSL) as natpool,
            tc.tile_pool(name="obpool", bufs=3) as obpool,
            tc.tile_pool(name="pst", bufs=4, space="PSUM") as pst,
        ):
            # input slice 0 rides the gpsimd ring: its preamble is ~1us
            # shorter than sync's (no HWDGE drain), so first data lands
            # earlier. Remaining slices go on sync.
            natf0 = natpool.tile([128, HSL * 2 * B], f16)
            nc.gpsimd.dma_start(out=natf0, in_=xt[:, 0 : HSL * 2 * B])
            nats = [natf0]

            wc_t = wcpool.tile([32, H_CORE * 32], f16)
            nc.gpsimd.dma_start(out=wc_t, in_=wc[:, :])

            # kron-expand wc -> wk_all[p, (h, c)] = I8 x W[h].T, BEFORE the
            # input prefetch so its DMA-completion wait doesn't get ordered
            # behind the whole input stream. Engine APs need 32-aligned
            # partition starts: per h-quarter, memset then copy kron(I2, Wt)
            # into the four diagonal 32-quadrants, so matmuls on early
            # channels unblock before the whole table is built. Vector and
            # gpsimd each own two h-quarters; scalar is kept off the
            # critical path (its first op pays a ~1.3us ACT_TABLE_LOAD).
            wk_all = wpool.tile([128, H_CORE * 128], f16)
            wk_v = wk_all.rearrange("p (h c) -> p h c", c=128)
            wc_v = wc_t.rearrange("g (h o) -> g h o", o=32)
            half = (H_CORE // 2) * 128
            nc.gpsimd.memset(wk_all[:, 0:half], 0.0)
            nc.vector.memset(wk_all[:, half:], 0.0)
            for q in range(4):
                nc.vector.tensor_copy(
                    out=wk_v[q * 32 : (q + 1) * 32, :, q * 32 : (q + 1) * 32],
                    in_=wc_v,
                )

            # prefetch the rest of the input; queues stream while the
            # weight expansion runs on the compute engines
            for hs in range(1, NSL):
                natf = natpool.tile([128, HSL * 2 * B], f16)
                nc.sync.dma_start(
                    out=natf,
                    in_=xt[:, hs * HSL * 2 * B : (hs + 1) * HSL * 2 * B],
                )
                nats.append(natf)

            for hs in range(NSL):
                natf = nats[hs]
                ob = obpool.tile([128, HSL * 2 * B], f16)
                for hp in range(HSL // 2):
                    # two matmuls share a 2-bank PSUM tile; one wide copy
                    # (gpsimd cannot read PSUM, so vector/scalar alternate)
                    ps = pst.tile([128, 1024], f32)
                    for d in range(2):
                        hl = hp * 2 + d
                        h = hs * HSL + hl
                        nc.tensor.matmul(
                            ps[:, d * 512 : (d + 1) * 512],
                            wk_all[:, h * 128 : (h + 1) * 128],
                            natf[:, hl * 512 : (hl + 1) * 512],
                            start=True,
                            stop=True,
                        )
                    dst = ob[:, hp * 1024 : (hp + 1) * 1024]
                    if (hs * 4 + hp) % 2 == 0:
                        nc.vector.tensor_copy(out=dst, in_=ps)
                    else:
                        nc.scalar.copy(dst, ps)
                # stores ride the sync ring, which is idle once the input
                # triggers are out; the last slice is split per h-pair so
                # the kernel tail drains as each paired copy lands
                if hs < NSL - 1:
                    nc.sync.dma_start(
                        out=yt[:, hs * HSL * 2 * B : (hs + 1) * HSL * 2 * B],
                        in_=ob,
                    )
                else:
                    for f in range(HSL // 2):
                        lo = (hs * HSL + f * 2) * 2 * B
                        hi = (hs * HSL + (f + 1) * 2) * 2 * B
                        eng = nc.sync if f % 2 == 0 else nc.gpsimd
                        eng.dma_start(
                            out=yt[:, lo:hi],
                            in_=ob[:, f * 2 * 2 * B : (f + 1) * 2 * 2 * B],
                        )

    nc.finalize()
    return nc


def _pack_weights(W: np.ndarray) -> np.ndarray:
    """Per-core wc [32, H_CORE*32] fp16: wc[:, h, :] = kron(I2, W[h].T)."""
    eye2 = np.eye(2, dtype=np.float32)
    wcs = np.empty((N_CORES, 32, H_CORE, 32), dtype=np.float16)
    for c in range(N_CORES):
        for h in range(H_CORE):
            Wt = W[c * H_CORE + h].T.astype(np.float32)
            wcs[c, :, h, :] = np.kron(eye2, Wt).astype(np.float16)
    return wcs.reshape(N_CORES, 32, H_CORE * 32)


def _get_bass():
    global _cached
    if _cached is None:
        _cached = _build_bass()
    return _cached


def kernel(x: np.ndarray, W: np.ndarray, _trace: bool = False):
    from concourse.bass_utils import run_bass_kernel_spmd

    nc = _get_bass()
    xh = np.asarray(x, dtype=np.float32).reshape(B, HIDDEN, SEQ).astype(np.float16)
    wcs = _pack_weights(np.asarray(W, dtype=np.float32))

    in_maps = []
    for c in range(N_CORES):
        # [b, h, sh, p] -> [p, h, sh, b], flattened to [128, H_CORE*2*B]
        xc = xh[:, c * H_CORE : (c + 1) * H_CORE, :].reshape(B, H_CORE, 2, 128)
        xc = np.ascontiguousarray(xc.transpose(3, 1, 2, 0)).reshape(128, -1)
        in_maps.append({"xt": xc, "wc": wcs[c]})

    res = run_bass_kernel_spmd(
        nc, in_maps, core_ids=list(range(N_CORES)), trace=_trace
    )
    parts = []
    for c in range(N_CORES):
        ytc = res.results[c]["yt"].reshape(128, H_CORE, 2, B)
        # yt[p, h, sh, b] -> y[b, h, sh*128+p]
        parts.append(ytc.transpose(3, 1, 2, 0).reshape(B, H_CORE, SEQ))
    out = np.concatenate(parts, axis=1).astype(np.float32)
    out = out.reshape(B, HIDDEN, 1, SEQ)
    if _trace:
        kernel._last_results = res
    return out


# revision 28
# speedup vs baseline: 1.1686x; 1.1686x over previous
"""Trainium2 Bass kernel for nn_LocalMixer: grouped 16x16 mixing conv.

out[b, h, t*16+go] = sum_gi W[h, go, gi] * x[b, h, t*16+gi]

Scheme: shard HIDDEN across the 8 cores (64 channels each, all 256
batches). The kernel is HBM-bound, so all layout work is on the host
and all device I/O is fp16:

  - x uploaded PRE-TRANSPOSED fp16: xt[p, h, sh, b] = x[b, h, sh*128+p].
    Partition p = seq pos within a 128-half = (t8, gi), exactly the PE
    contraction layout -> no on-device transposes.
  - W uploaded COMPACT fp16 (wc[gi, h, go], 32 KB) and kron-expanded on
    device into wk[p=(t8,gi), h, (t8',go)] = I8(t8,t8') * W[h,go,gi]
    via memset + 8 strided copies -- keeps 2.1 MB of block-diag zeros
    off HBM.
  - Per h: ONE matmul, lhsT = wk[:, h, :] (stationary 128x128 kron
    block), rhs = xt[:, h, :, :] (all 512 (sh, b) columns stream) ->
    PSUM [(t8, go), (sh, b)] f32.
  - PSUM -> fp16 SBUF copy (vector/scalar alternating), stored to yt in
    the transposed layout yt[p, h, sh, b]; the host un-transposes and
    upcasts to f32 for free.

Device traffic: 8.39 (in) + 8.39 (out) + 0.03 (W) MB per core at fp16,
vs 33.6 MB for the f32 version. Rel err ~3e-4 (fp16 operand rounding;
f32 PSUM accumulate).
"""

import numpy as np

B = 256
HIDDEN = 512
BLOCK = 16
GROUPS = 16
SEQ = BLOCK * GROUPS  # 256
N_CORES = 8
H_CORE = HIDDEN // N_CORES  # 64 hidden channels per core
HSL = 8  # h channels per input/output slice
NSL = H_CORE // HSL  # 8 slices

_cached = None


def _build_bass():
    import concourse.mybir as mybir
    from concourse import bacc
    from concourse.tile import TileContext

    f32 = mybir.dt.float32
    f16 = mybir.dt.float16
    nc = bacc.Bacc()
    # xt layout: [p, (h, sh, b)] with p = seq pos within 128-half
    xt = nc.declare_dram_parameter("xt", [128, H_CORE * 2 * B], f16, isOutput=False)
    # pair-expanded weights: wc[p2, (h, c2)] = kron(I2, W[h].T)
    wc = nc.declare_dram_parameter("wc", [32, H_CORE * 32], f16, isOutput=False)
    # yt layout: [p, (h, sh, b)] like xt; host un-transposes
    yt = nc.declare_dram_parameter("yt", [128, H_CORE * 2 * B], f16, isOutput=True)

    with TileContext(nc) as tc:
        with (
            tc.tile_pool(name="wpool", bufs=1) as wpool,
            tc.tile_pool(name="wcpool", bufs=1) as wcpool,
            tc.tile_pool(name="natpool", bufs=NSL) as natpool,
            tc.tile_pool(name="obpool", bufs=2) as obpool,
            tc.tile_pool(name="pst", bufs=4, space="PSUM") as pst,
        ):
            wc_t = wcpool.tile([32, H_CORE * 32], f16)
            nc.gpsimd.dma_start(out=wc_t, in_=wc[:, :])

            # kron-expand wc -> wk_all[p, (h, c)] = I8 x W[h].T, BEFORE the
            # input prefetch so its DMA-completion wait doesn't get ordered
            # behind the whole input stream. Engine APs need 32-aligned
            # partition starts: per h-quarter, memset then copy kron(I2, Wt)
            # into the four diagonal 32-quadrants, so matmuls on early
            # channels unblock before the whole table is built. Vector and
            # gpsimd each own two h-quarters; scalar is kept off the
            # critical path (its first op pays a ~1.3us ACT_TABLE_LOAD).
            wk_all = wpool.tile([128, H_CORE * 128], f16)
            wk_v = wk_all.rearrange("p (h c) -> p h c", c=128)
            wc_v = wc_t.rearrange("g (h o) -> g h o", o=32)
            half = (H_CORE // 2) * 128
            nc.gpsimd.memset(wk_all[:, 0:half], 0.0)
            nc.vector.memset(wk_all[:, half:], 0.0)
            for q in range(4):
                nc.vector.tensor_copy(
                    out=wk_v[q * 32 : (q + 1) * 32, :, q * 32 : (q + 1) * 32],
                    in_=wc_v,
                )

            # prefetch the whole input; queues stream while the weight
            # expansion runs on the compute engines
            nats = []
            for hs in range(NSL):
                natf = natpool.tile([128, HSL * 2 * B], f16)
                nc.sync.dma_start(
                    out=natf,
                    in_=xt[:, hs * HSL * 2 * B : (hs + 1) * HSL * 2 * B],
                )
                nats.append(natf)

            # each ob tile spans TWO input slices (16 h): store rows are
            # then 16 KiB, which the write path sustains at ~2x the
            # per-queue throughput of 8 KiB rows
            ob = None
            for hs in range(NSL):
                natf = nats[hs]
                if hs % 2 == 0:
                    ob = obpool.tile([128, 2 * HSL * 2 * B], f16)
                og = (hs % 2) * HSL * 2 * B
                for hp in range(HSL // 2):
                    # two matmuls share a 2-bank PSUM tile; one wide copy
                    # (gpsimd cannot read PSUM, so vector/scalar alternate)
                    ps = pst.tile([128, 1024], f32)
                    for d in range(2):
                        hl = hp * 2 + d
                        h = hs * HSL + hl
                        nc.tensor.matmul(
                            ps[:, d * 512 : (d + 1) * 512],
                            wk_all[:, h * 128 : (h + 1) * 128],
                            natf[:, hl * 512 : (hl + 1) * 512],
                            start=True,
                            stop=True,
                        )
                    dst = ob[:, og + hp * 1024 : og + (hp + 1) * 1024]
                    if (hs * 4 + hp) % 2 == 0:
                        nc.vector.tensor_copy(out=dst, in_=ps)
                    else:
                        nc.scalar.copy(dst, ps)
                # stores ride the sync ring, which is idle once the input
                # triggers are out
                if hs % 2 == 1:
                    nc.sync.dma_start(
                        out=yt[:, (hs - 1) * HSL * 2 * B : (hs + 1) * HSL * 2 * B],
                        in_=ob,
                    )

    nc.finalize()
    return nc


def _pack_weights(W: np.ndarray) -> np.ndarray:
    """Per-core wc [32, H_CORE*32] fp16: wc[:, h, :] = kron(I2, W[h].T)."""
    eye2 = np.eye(2, dtype=np.float32)
    wcs = np.empty((N_CORES, 32, H_CORE, 32), dtype=np.float16)
    for c in range(N_CORES):
        for h in range(H_CORE):
            Wt = W[c * H_CORE + h].T.astype(np.float32)
            wcs[c, :, h, :] = np.kron(eye2, Wt).astype(np.float16)
    return wcs.reshape(N_CORES, 32, H_CORE * 32)


def _get_bass():
    global _cached
    if _cached is None:
        _cached = _build_bass()
    return _cached


def kernel(x: np.ndarray, W: np.ndarray, _trace: bool = False):
    from concourse.bass_utils import run_bass_kernel_spmd

    nc = _get_bass()
    xh = np.asarray(x, dtype=np.float32).reshape(B, HIDDEN, SEQ).astype(np.float16)
    wcs = _pack_weights(np.asarray(W, dtype=np.float32))

    in_maps = []
    for c in range(N_CORES):
        # [b, h, sh, p] -> [p, h, sh, b], flattened to [128, H_CORE*2*B]
        xc = xh[:, c * H_CORE : (c + 1) * H_CORE, :].reshape(B, H_CORE, 2, 128)
        xc = np.ascontiguousarray(xc.transpose(3, 1, 2, 0)).reshape(128, -1)
        in_maps.append({"xt": xc, "wc": wcs[c]})

    res = run_bass_kernel_spmd(
        nc, in_maps, core_ids=list(range(N_CORES)), trace=_trace
    )
    parts = []
    for c in range(N_CORES):
        ytc = res.results[c]["yt"].reshape(128, H_CORE, 2, B)
        # yt[p, h, sh, b] -> y[b, h, sh*128+p]
        parts.append(ytc.transpose(3, 1, 2, 0).reshape(B, H_CORE, SEQ))
    out = np.concatenate(parts, axis=1).astype(np.float32)
    out = out.reshape(B, HIDDEN, 1, SEQ)
    if _trace:
        kernel._last_results = res
    return out
